# revision 17
# baseline (speedup 1.0000x reference)
"""Trainium2 Bass kernel for BoundaryConvLayer GNN message passing.

Strategy (8 NeuronCores, no collectives):
  - Nodes assigned to 8 cores x windows of 128 slots, balanced by in-degree.
  - x replicated (bf16) per core as DRAM gather table; each core DMA-gathers
    x[src] rows (256B) for its own edges via SWDGE dma_gather.  Gather streams
    are packed RAGGED per (8-window group, 32k src chunk): edges sorted by
    window, padded to 128 only at chunk-call boundaries (the SWDGE desc-gen at
    ~9.5ns/idx is the hard bottleneck, so padding is minimized).  Tile
    geometry (tile counts, window tile ranges) is the max/union over the 8
    cores so a single SPMD program serves all cores.
  - Scatter-reduce per dst window via one-hot matmuls: S built on DVE with
    is_equal against iota; boundary tiles shared by two windows use per-window
    masked slot streams (255 = no match).
  - Per-node MLPs in bf16; Softplus/Rsqrt/Gelu on ACT (batched per function to
    minimize ACT table loads); LayerNorm stats via bn_stats, applies on DVE.
  - Output written per-core, inverse-permuted on host.
"""

import sys

sys.path.insert(0, "/opt/trn_rl_repo")

import heapq

import ml_dtypes
import numpy as np

from concourse import bacc, bass, tile
from concourse.bass_utils import run_bass_kernel_spmd

mybir = bass.mybir
f32 = mybir.dt.float32
bf16 = mybir.dt.bfloat16
u8 = mybir.dt.uint8
i16 = mybir.dt.int16

P = 128
N_CORES = 8
TRACE = False
LAST_EXEC_TIME_NS = None
CHUNK = 32768  # int16 gather-index range per chunk
GROUP_SUPERS = 1  # supers (of up to 4 windows) per gather group
EPS = 1e-4
LN_EPS = 1e-5


# ----------------------------------------------------------------------------
# host-side planning
# ----------------------------------------------------------------------------

def _balanced_assignment(indeg, n_slots):
    """Assign node ids (len(indeg) <= n_slots) to n_slots//128 windows of 128
    slots each, minimizing max window edge-load. Returns slot_of_node."""
    n_win = n_slots // P
    n = len(indeg)
    order = np.argsort(-indeg, kind="stable")
    slot_of_node = np.empty(n_slots, dtype=np.int64)
    heap = [(0.0, w) for w in range(n_win)]
    heapq.heapify(heap)
    counts = np.zeros(n_win, dtype=np.int64)
    ids = np.concatenate([order, np.arange(n, n_slots)])
    degs = np.concatenate([indeg[order], np.zeros(n_slots - n, dtype=indeg.dtype)])
    for i in range(n_slots):
        while True:
            load, w = heapq.heappop(heap)
            if counts[w] < P:
                break
        slot_of_node[ids[i]] = w * P + counts[w]
        counts[w] += 1
        heapq.heappush(heap, (load + float(degs[i]), w))
    return slot_of_node


def _plan(x, edge_index, degree):
    N, D = x.shape
    assert D == P
    E = edge_index.shape[1]
    spc = -(-N // (N_CORES * P)) * P           # node slots per core
    n_win = spc // P                            # windows per core
    n_slots = spc * N_CORES
    NC = -(-N // CHUNK)                         # src chunks

    # supers (MLP tiles of up to 4 windows), groups (gather units of up to
    # GROUP_SUPERS supers)
    supers = []
    w0 = 0
    while w0 < n_win:
        W = min(4, n_win - w0)
        supers.append((w0, W))
        w0 += W
    groups = []
    si = 0
    while si < len(supers):
        gw0 = supers[si][0]
        nw = supers[si][1]
        sis = [si]
        for k in range(1, GROUP_SUPERS):
            if si + k < len(supers):
                nw += supers[si + k][1]
                sis.append(si + k)
        groups.append((gw0, nw, sis))
        si += GROUP_SUPERS
    NG = len(groups)
    grp_of_win = np.zeros(n_win, dtype=np.int64)
    for gi, (gw0, nw, _) in enumerate(groups):
        grp_of_win[gw0:gw0 + nw] = gi

    src = np.asarray(edge_index[0], dtype=np.int64)
    dst = np.asarray(edge_index[1], dtype=np.int64)
    indeg = np.bincount(dst, minlength=N)
    slot_of_node = _balanced_assignment(indeg, n_slots)
    node_of_slot = np.empty(n_slots, dtype=np.int64)
    node_of_slot[slot_of_node] = np.arange(n_slots)

    gslot = slot_of_node[dst]
    core = gslot // spc
    pos = gslot % spc
    gw = pos >> 7                    # window within core
    lane = pos & 127                 # dst slot within window
    grp = grp_of_win[gw]
    chunk = src // CHUNK
    srcloc = (src % CHUNK).astype(np.int16)

    # sort edges by (core, grp, chunk, window); ranks within (core, grp, chunk)
    key_gc = (core * NG + grp) * NC + chunk            # stream id
    order = np.lexsort((gw, chunk, grp, core))
    key_s = key_gc[order]
    first = np.ones(E, dtype=bool)
    first[1:] = key_s[1:] != key_s[:-1]
    starts = np.flatnonzero(first)
    seg_of = np.cumsum(first) - 1
    rank_s = np.arange(E) - starts[seg_of]             # rank in sorted order

    rank = np.empty(E, dtype=np.int64)
    rank[order] = rank_s

    # per (core, g, c) counts -> shared tile counts NT[g, c]
    cnt = np.bincount(key_gc, minlength=N_CORES * NG * NC).reshape(N_CORES, NG, NC)
    NT = -(-cnt.max(axis=0) // P)                      # [NG, NC]
    off = np.zeros((NG, NC), dtype=np.int64)
    for g in range(NG):
        o = 0
        for c in range(NC):
            off[g, c] = o
            o += NT[g, c]
    NT_g = NT.sum(axis=1)                              # [NG]
    idx_base = np.zeros(NG, dtype=np.int64)
    idx_base[1:] = np.cumsum(NT_g * P)[:-1]
    total_idx = int((NT_g * P).sum())

    # per (core, g, c, w) first/last rank -> union tile range over cores
    key_w = key_gc * n_win + gw
    key_w_s = key_w[order]
    uk, ufirst, ucnt = np.unique(key_w_s, return_index=True, return_counts=True)
    r_first = rank_s[ufirst]
    r_last = rank_s[ufirst + ucnt - 1]
    t0 = np.full((NG, NC, n_win), 2**30, dtype=np.int64)
    t1 = np.full((NG, NC, n_win), -1, dtype=np.int64)
    uw = uk % n_win
    ukgc = uk // n_win
    uc = ukgc % NC
    ug = (ukgc // NC) % NG
    np.minimum.at(t0, (ug, uc, uw), r_first >> 7)
    np.maximum.at(t1, (ug, uc, uw), r_last >> 7)

    # per-window matmul maps (G columns) and slot-stream layout
    gcols = [None] * n_win       # window -> list of G-tile column indices
    rw = np.zeros(n_win, dtype=np.int64)
    rng_off = np.zeros((NG, NC, n_win), dtype=np.int64)  # jj offset of (g,c,w)
    for w in range(n_win):
        g = grp_of_win[w]
        cols = []
        for c in range(NC):
            if t1[g, c, w] >= 0:
                rng_off[g, c, w] = len(cols)
                for t in range(t0[g, c, w], t1[g, c, w] + 1):
                    cols.append(off[g, c] + t)
            else:
                rng_off[g, c, w] = -1
        gcols[w] = cols
        rw[w] = len(cols)
    slot_base = np.zeros(n_win, dtype=np.int64)
    slot_base[1:] = np.cumsum(rw)[:-1]
    total_rw = int(rw.sum())

    # build per-core streams
    idx_streams = np.zeros((N_CORES, total_idx), dtype=np.int16)
    e_pos = idx_base[grp] + off[grp, chunk] * P + rank
    idx_streams[core, e_pos] = srcloc

    slots = np.full((N_CORES, total_rw, P), 255, dtype=np.uint8)
    e_jj = slot_base[gw] + rng_off[grp, chunk, gw] + (rank >> 7) - t0[grp, chunk, gw]
    slots[core, e_jj, rank & 127] = lane

    plan = dict(
        N=N, D=D, E=E, spc=spc, n_win=n_win, NC=NC, NG=NG,
        supers=supers, groups=groups,
        NT=NT, off=off, NT_g=NT_g, idx_base=idx_base,
        gcols=gcols, rw=rw, slot_base=slot_base,
        total_idx=total_idx, total_rw=total_rw,
        slot_of_node=slot_of_node, node_of_slot=node_of_slot,
        idx_streams=idx_streams, slots=slots,
    )
    return plan


# ----------------------------------------------------------------------------
# device program
# ----------------------------------------------------------------------------

def _build_program(plan, flags):
    n_win = plan["n_win"]
    NC, NG = plan["NC"], plan["NG"]
    supers, groups = plan["supers"], plan["groups"]
    NT, off, NT_g = plan["NT"], plan["off"], plan["NT_g"]
    idx_base = plan["idx_base"]
    gcols, rw, slot_base = plan["gcols"], plan["rw"], plan["slot_base"]
    spc = plan["spc"]
    N = plan["N"]
    total_idx, total_rw = plan["total_idx"], plan["total_rw"]
    H = flags["H"]
    nH = H // P

    NT_CAP = int(NT_g.max())
    RW_CAP = int(rw.max())
    # slots columns per super
    sup_sl = []
    for (w0, W) in supers:
        sup_sl.append((int(slot_base[w0]), int(rw[w0:w0 + W].sum())))
    SLW_CAP = max(s[1] for s in sup_sl)

    nc = bacc.Bacc("TRN2", target_bir_lowering=False)

    xg_d = nc.dram_tensor("xg", [N, P], bf16, kind="ExternalInput")
    xt_d = nc.dram_tensor("xt", [P, spc], bf16, kind="ExternalInput")
    xf_d = nc.dram_tensor("xf", [spc, P], f32, kind="ExternalInput")
    deg_d = nc.dram_tensor("deg", [P, n_win], f32, kind="ExternalInput")
    idx_d = nc.dram_tensor("idx", [P, total_idx // 16], i16, kind="ExternalInput")
    slots_d = nc.dram_tensor("slots", [P, total_rw], u8, kind="ExternalInput")
    iota_d = nc.dram_tensor("iota", [P, P], u8, kind="ExternalInput")
    Wr_d = nc.dram_tensor("Wr", [P, P], bf16, kind="ExternalInput")
    Wb1_d = nc.dram_tensor("Wb1", [P, H], bf16, kind="ExternalInput")
    W1_d = nc.dram_tensor("W1", [P, H], bf16, kind="ExternalInput")
    Wb2c_d = nc.dram_tensor("Wb2c", [P, nH, P], bf16, kind="ExternalInput")
    W2c_d = nc.dram_tensor("W2c", [P, nH, P], bf16, kind="ExternalInput")
    bb1c_d = nc.dram_tensor("bb1c", [P, nH], f32, kind="ExternalInput")
    b1c_d = nc.dram_tensor("b1c", [P, nH], f32, kind="ExternalInput")
    ident_d = nc.dram_tensor("ident", [P, P], bf16, kind="ExternalInput")
    bcast_names = [n for n in ("brb", "bb2b", "grbb", "brbb", "gnb", "bnb")
                   if flags[n]]
    bcast_d = {n: nc.dram_tensor(n, [P, P], f32, kind="ExternalInput")
               for n in bcast_names}
    out_d = nc.dram_tensor("out", [spc, P], f32, kind="ExternalOutput")

    AT = mybir.ActivationFunctionType
    OP = mybir.AluOpType

    with tile.TileContext(nc) as tc:
        with tc.tile_pool(name="const", bufs=1) as cp, \
             tc.tile_pool(name="stream", bufs=3) as sp, \
             tc.tile_pool(name="gat", bufs=3) as gp, \
             tc.tile_pool(name="sS", bufs=6) as ssp, \
             tc.tile_pool(name="work", bufs=2) as wp, \
             tc.tile_pool(name="tiny", bufs=8) as tp, \
             tc.tile_pool(name="ps_big", bufs=2, space="PSUM") as ps_big, \
             tc.tile_pool(name="ps_z", bufs=2, space="PSUM") as ps_z, \
             tc.tile_pool(name="ps_agg", bufs=2, space="PSUM") as ps_agg, \
             tc.tile_pool(name="ps_ht", bufs=1, space="PSUM") as ps_ht, \
             tc.tile_pool(name="ps_out", bufs=1, space="PSUM") as ps_out:

            # ---- constants
            def cload(dram, shape, dtype, tag):
                t = cp.tile(shape, dtype, tag=tag)
                nc.scalar.dma_start(out=t[:], in_=dram[:])
                return t

            eps_t = cp.tile([P, 1], f32, tag="eps")
            nc.vector.memset(eps_t[:], LN_EPS)

            Wr_t = cload(Wr_d, [P, P], bf16, "Wr")
            Wb1_t = cload(Wb1_d, [P, H], bf16, "Wb1")
            W1_t = cload(W1_d, [P, H], bf16, "W1")
            Wb2c_t = cload(Wb2c_d, [P, nH, P], bf16, "Wb2c")
            W2c_t = cload(W2c_d, [P, nH, P], bf16, "W2c")
            bb1c_t = cload(bb1c_d, [P, nH], f32, "bb1c")
            b1c_t = cload(b1c_d, [P, nH], f32, "b1c")
            ident_t = cload(ident_d, [P, P], bf16, "ident")
            iota_t = cload(iota_d, [P, P], u8, "iota")
            deg_t = cload(deg_d, [P, n_win], f32, "deg")
            bcast_t = {n: cload(bcast_d[n], [P, P], f32, n) for n in bcast_names}

            for g, (gw0, gnw, sis) in enumerate(groups):
                ntg = int(NT_g[g])
                # ---- group streams: gather indices + gathered rows
                idx_t = sp.tile([P, NT_CAP * 8], i16, tag="idx")
                nc.sync.dma_start(
                    out=idx_t[:, :ntg * 8],
                    in_=idx_d[:, idx_base[g] // 16: (idx_base[g] + ntg * P) // 16])
                Gt = gp.tile([P, NT_CAP, P], bf16, tag="G")
                for c in range(NC):
                    n_t = int(NT[g, c])
                    if n_t == 0:
                        continue
                    lo = c * CHUNK
                    hi = min(N, lo + CHUNK)
                    o = int(off[g, c])
                    nc.gpsimd.dma_gather(
                        out_ap=Gt[:, o: o + n_t, :],
                        in_ap=xg_d[lo:hi, :],
                        idxs_ap=idx_t[:, o * 8: (o + n_t) * 8],
                        num_idxs=n_t * P, num_idxs_reg=n_t * P, elem_size=P,
                        single_packet=False)

                for si in sis:
                    w0, W = supers[si]
                    R = W * P
                    # ---- super streams
                    xt_s = sp.tile([P, 4 * P], bf16, tag="xt")
                    nc.sync.dma_start(out=xt_s[:, :R],
                                      in_=xt_d[:, w0 * P: w0 * P + R])
                    xf_s = sp.tile([P, 4, P], f32, tag="xf")
                    nc.sync.dma_start(
                        out=xf_s[:, :W, :],
                        in_=xf_d[w0 * P: w0 * P + R, :].rearrange(
                            "(w p) f -> p w f", p=P))
                    sl0, slw = sup_sl[si]
                    slots_t = sp.tile([P, SLW_CAP], u8, tag="slots")
                    nc.sync.dma_start(out=slots_t[:, :slw],
                                      in_=slots_d[:, sl0: sl0 + slw])

                    # ---- S one-hots (DVE) per window
                    Ss = []
                    so = 0
                    for wl in range(W):
                        rww = int(rw[w0 + wl])
                        S = ssp.tile([P, RW_CAP, P], bf16, tag="S")
                        if rww:
                            sb = slots_t[:, so: so + rww].broadcast_to(
                                [P, rww, P])
                            ib = iota_t[:].rearrange(
                                "p (o f) -> p o f", o=1).broadcast_to([P, rww, P])
                            nc.vector.tensor_tensor(out=S[:, :rww, :], in0=sb,
                                                    in1=ib, op=OP.is_equal)
                        Ss.append(S)
                        so += rww

                    # ---- rob_bound layer 1 (A-pattern) + rate.
                    # softplus(u) = -ln(sigmoid(-u)); we keep the NEGATED
                    # value (ln(sigmoid(-u))) and let the sign ride through.
                    # ACT sequence is batched by function: Sig x5, Ln x2.
                    g1T = wp.tile([P, nH, 4 * P], bf16, tag="g1T")
                    for cH in range(nH):
                        pb = ps_big.tile([P, 4 * P], f32, tag="bigA")
                        nc.tensor.matmul(pb[:, :R], Wb1_t[:, cH * P:(cH + 1) * P],
                                         xt_s[:, :R], start=True, stop=True)
                        nc.scalar.activation(g1T[:, cH, :R], pb[:, :R],
                                             AT.Sigmoid,
                                             bias=bb1c_t[:, cH:cH + 1],
                                             scale=-1.0)
                    # lr = ln(sigmoid(-(x@Wr+br))) = -rate
                    pr = ps_z.tile([P, 4, P], f32, tag="z")
                    for wl in range(W):
                        nc.tensor.matmul(pr[:, wl, :],
                                         xt_s[:, wl * P:(wl + 1) * P],
                                         Wr_t[:], start=True, stop=True)
                    lr = wp.tile([P, 4, P], f32, tag="rate")
                    if flags["brb"]:
                        br_b = bcast_t["brb"][:].rearrange(
                            "p (o f) -> p o f", o=1).broadcast_to([P, W, P])
                        nc.vector.tensor_tensor(out=lr[:, :W, :],
                                                in0=pr[:, :W, :], in1=br_b,
                                                op=OP.add)
                        nc.scalar.activation(lr[:, :W, :], lr[:, :W, :],
                                             AT.Sigmoid, bias=0.0, scale=-1.0)
                    else:
                        nc.scalar.activation(lr[:, :W, :], pr[:, :W, :],
                                             AT.Sigmoid, bias=0.0, scale=-1.0)
                    # Ln passes (one table load): g1T in-place, lr in-place
                    if R == 4 * P:
                        nc.scalar.activation(
                            g1T[:].rearrange("p c r -> p (c r)"),
                            g1T[:].rearrange("p c r -> p (c r)"),
                            AT.Ln, bias=0.0, scale=1.0)
                    else:
                        for cH in range(nH):
                            nc.scalar.activation(g1T[:, cH, :R], g1T[:, cH, :R],
                                                 AT.Ln, bias=0.0, scale=1.0)
                    nc.scalar.activation(lr[:, :W, :], lr[:, :W, :],
                                         AT.Ln, bias=0.0, scale=1.0)

                    # ---- rob_bound layer 2 (B-pattern) -> pz[rows, feat]
                    pz = ps_z.tile([P, 4, P], f32, tag="z")
                    for wl in range(W):
                        for cH in range(nH):
                            nc.tensor.matmul(
                                pz[:, wl, :],
                                g1T[:, cH, wl * P:(wl + 1) * P],
                                Wb2c_t[:, cH, :],
                                start=(cH == 0), stop=(cH == nH - 1))

                    z_in = pz
                    if flags["bb2b"]:
                        z_sb = wp.tile([P, 4, P], f32, tag="z_sb")
                        bb2_b = bcast_t["bb2b"][:].rearrange(
                            "p (o f) -> p o f", o=1).broadcast_to([P, W, P])
                        nc.vector.tensor_tensor(out=z_sb[:, :W, :],
                                                in0=pz[:, :W, :], in1=bb2_b,
                                                op=OP.subtract)
                        z_in = z_sb

                    # ---- LN stats (DVE) for z and x; rsqrt batched on ACT
                    mvz = tp.tile([P, 2 * 4], f32, tag="mvz")
                    mvx = tp.tile([P, 2 * 4], f32, tag="mvx")
                    for wl in range(W):
                        stz = tp.tile([P, 6], f32, tag="stz")
                        nc.vector.bn_stats(stz[:], z_in[:, wl, :])
                        nc.vector.bn_aggr(mvz[:, 2 * wl:2 * wl + 2], stz[:])
                        stx = tp.tile([P, 6], f32, tag="stx")
                        nc.vector.bn_stats(stx[:], xf_s[:, wl, :])
                        nc.vector.bn_aggr(mvx[:, 2 * wl:2 * wl + 2], stx[:])
                    sdz = tp.tile([P, 4], f32, tag="sdz")
                    sdx = tp.tile([P, 4], f32, tag="sdx")
                    for wl in range(W):
                        nc.scalar.activation(sdz[:, wl:wl + 1],
                                             mvz[:, 2 * wl + 1:2 * wl + 2],
                                             AT.Sqrt, bias=eps_t[:, 0:1],
                                             scale=1.0)
                        nc.scalar.activation(sdx[:, wl:wl + 1],
                                             mvx[:, 2 * wl + 1:2 * wl + 2],
                                             AT.Sqrt, bias=eps_t[:, 0:1],
                                             scale=1.0)
                    rsz = tp.tile([P, 4], f32, tag="rsz")
                    rsx = tp.tile([P, 4], f32, tag="rsx")
                    nc.vector.reciprocal(rsz[:, :W], sdz[:, :W])
                    nc.vector.reciprocal(rsx[:, :W], sdx[:, :W])
                    mbz = tp.tile([P, 4], f32, tag="mbz")
                    mbx = tp.tile([P, 4], f32, tag="mbx")
                    for wl in range(W):
                        nc.vector.tensor_scalar(
                            out=mbz[:, wl:wl + 1], in0=mvz[:, 2 * wl:2 * wl + 1],
                            scalar1=rsz[:, wl:wl + 1], scalar2=-1.0,
                            op0=OP.mult, op1=OP.mult)
                        nc.vector.tensor_scalar(
                            out=mbx[:, wl:wl + 1], in0=mvx[:, 2 * wl:2 * wl + 1],
                            scalar1=rsx[:, wl:wl + 1], scalar2=-1.0,
                            op0=OP.mult, op1=OP.mult)

                    # gamma' = -LN_core(z) (apply on DVE; pz holds -z so the
                    # normalized value comes out negated), then optional affine
                    gamma = wp.tile([P, 4, P], f32, tag="gamma")
                    for wl in range(W):
                        nc.vector.tensor_scalar(
                            out=gamma[:, wl, :], in0=z_in[:, wl, :],
                            scalar1=rsz[:, wl:wl + 1], scalar2=mbz[:, wl:wl + 1],
                            op0=OP.mult, op1=OP.add)
                    gamma_negated = True
                    if flags["grbb"]:
                        g_b = bcast_t["grbb"][:].rearrange(
                            "p (o f) -> p o f", o=1).broadcast_to([P, W, P])
                        nc.vector.scalar_tensor_tensor(
                            out=gamma[:, :W, :], in0=gamma[:, :W, :],
                            scalar=-1.0, in1=g_b, op0=OP.mult, op1=OP.mult)
                        gamma_negated = False
                    if flags["brbb"]:
                        b_b = bcast_t["brbb"][:].rearrange(
                            "p (o f) -> p o f", o=1).broadcast_to([P, W, P])
                        if gamma_negated:
                            nc.vector.scalar_tensor_tensor(
                                out=gamma[:, :W, :], in0=gamma[:, :W, :],
                                scalar=-1.0, in1=b_b, op0=OP.mult, op1=OP.add)
                            gamma_negated = False
                        else:
                            nc.vector.tensor_tensor(out=gamma[:, :W, :],
                                                    in0=gamma[:, :W, :],
                                                    in1=b_b, op=OP.add)

                    # ---- agg: one-hot matmul accumulation per window
                    pagg = ps_agg.tile([P, 4, P], f32, tag="agg")
                    for wl in range(W):
                        cols = gcols[w0 + wl]
                        nj = len(cols)
                        if nj == 0:
                            nc.vector.memset(pagg[:, wl, :], 0.0)
                            continue
                        S = Ss[wl]
                        for jj, gcol in enumerate(cols):
                            nc.tensor.matmul(
                                pagg[:, wl, :],
                                S[:, jj, :],
                                Gt[:, gcol, :],
                                start=(jj == 0), stop=(jj == nj - 1))

                    # ---- h = (rate*agg + gamma) / (1 + rate*deg + EPS)
                    # rate = -lr ; gamma = -gamma' when gamma_negated
                    num = wp.tile([P, 4, P], f32, tag="num")
                    nc.vector.scalar_tensor_tensor(
                        out=num[:, :W, :], in0=lr[:, :W, :], scalar=-1.0,
                        in1=pagg[:, :W, :], op0=OP.mult, op1=OP.mult)
                    nc.vector.tensor_tensor(
                        out=num[:, :W, :], in0=num[:, :W, :],
                        in1=gamma[:, :W, :],
                        op=OP.subtract if gamma_negated else OP.add)
                    den = wp.tile([P, 4, P], f32, tag="den")
                    deg_b = deg_t[:, w0:w0 + W].rearrange(
                        "p (w o) -> p w o", o=1).broadcast_to([P, W, P])
                    nc.vector.scalar_tensor_tensor(
                        out=den[:, :W, :], in0=lr[:, :W, :], scalar=-1.0,
                        in1=deg_b, op0=OP.mult, op1=OP.mult)
                    nc.vector.tensor_scalar(out=den[:, :W, :],
                                            in0=den[:, :W, :],
                                            scalar1=1.0 + EPS, scalar2=None,
                                            op0=OP.add)
                    rcp = wp.tile([P, 4, P], f32, tag="rcp")
                    nc.vector.reciprocal_approx_fast(out=rcp[:, :W, :],
                                                     in_=den[:, :W, :])
                    h_bf = wp.tile([P, 4, P], bf16, tag="h_bf")
                    nc.vector.tensor_tensor(out=h_bf[:, :W, :],
                                            in0=num[:, :W, :],
                                            in1=rcp[:, :W, :], op=OP.mult)

                    # ---- hT via PE transpose
                    pht = ps_ht.tile([P, 4, P], bf16, tag="ht")
                    for wl in range(W):
                        nc.tensor.transpose(pht[:, wl, :], h_bf[:, wl, :],
                                            ident_t[:])
                    hT = wp.tile([P, 4 * P], bf16, tag="hT")
                    nc.vector.tensor_copy(
                        hT[:, :R], pht[:, :W, :].rearrange("p w f -> p (w f)"))

                    # ---- fc layer 1 (A-pattern on hT) + gelu
                    g2T = wp.tile([P, nH, 4 * P], bf16, tag="g2T")
                    for cH in range(nH):
                        pf = ps_big.tile([P, 4 * P], f32, tag="bigA")
                        nc.tensor.matmul(pf[:, :R], W1_t[:, cH * P:(cH + 1) * P],
                                         hT[:, :R], start=True, stop=True)
                        nc.scalar.activation(g2T[:, cH, :R], pf[:, :R], AT.Gelu,
                                             bias=b1c_t[:, cH:cH + 1], scale=1.0)

                    # ---- fc layer 2 (B-pattern) -> po[rows, feat]
                    po = ps_out.tile([P, 4, P], f32, tag="o")
                    for wl in range(W):
                        for cH in range(nH):
                            nc.tensor.matmul(
                                po[:, wl, :],
                                g2T[:, cH, wl * P:(wl + 1) * P],
                                W2c_t[:, cH, :],
                                start=(cH == 0), stop=(cH == nH - 1))

                    # ---- x_res = LN(x) applied on DVE; out = po + x_res
                    xres = wp.tile([P, 4, P], f32, tag="xres")
                    for wl in range(W):
                        nc.vector.tensor_scalar(
                            out=xres[:, wl, :], in0=xf_s[:, wl, :],
                            scalar1=rsx[:, wl:wl + 1], scalar2=mbx[:, wl:wl + 1],
                            op0=OP.mult, op1=OP.add)
                    if flags["gnb"]:
                        g_b = bcast_t["gnb"][:].rearrange(
                            "p (o f) -> p o f", o=1).broadcast_to([P, W, P])
                        nc.vector.tensor_tensor(out=xres[:, :W, :],
                                                in0=xres[:, :W, :], in1=g_b,
                                                op=OP.mult)
                    if flags["bnb"]:
                        b_b = bcast_t["bnb"][:].rearrange(
                            "p (o f) -> p o f", o=1).broadcast_to([P, W, P])
                        nc.vector.tensor_tensor(out=xres[:, :W, :],
                                                in0=xres[:, :W, :], in1=b_b,
                                                op=OP.add)

                    out_sb = wp.tile([P, 4, P], f32, tag="out_sb")
                    nc.vector.tensor_tensor(out=out_sb[:, :W, :],
                                            in0=po[:, :W, :],
                                            in1=xres[:, :W, :], op=OP.add)
                    nc.scalar.dma_start(
                        out=out_d[w0 * P: w0 * P + R, :].rearrange(
                            "(w p) f -> p w f", p=P),
                        in_=out_sb[:, :W, :])

    nc.compile()
    return nc


def _ensure_ntff_hook():
    """Register the axon NTFF profile hook when the container's antenv
    package lacks axon_hooks (needed for trace=True under axon)."""
    import types
    try:
        from antenv.axon_hooks import get_axon_ntff_profile_hook  # noqa: F401
        return
    except ImportError:
        pass
    if "/root/.axon_site" not in sys.path:
        sys.path.insert(0, "/root/.axon_site")
    from trn_agent_boot.trn_boot import _ntff_profile_via_ctypes
    import antenv
    hook = _ntff_profile_via_ctypes("/opt/axon/libaxon_pjrt.so")
    mod = types.ModuleType("antenv.axon_hooks")
    mod.get_axon_ntff_profile_hook = lambda: hook
    mod.set_axon_ntff_profile_hook = lambda h: None
    sys.modules["antenv.axon_hooks"] = mod
    antenv.axon_hooks = mod


# ----------------------------------------------------------------------------
# entry point
# ----------------------------------------------------------------------------

def kernel(x, edge_index, degree, Wr, br, Wb1, bb1, Wb2, bb2, g_rb, b_rb,
           W1, b1, W2, b2, g_n, b_n):
    x = np.asarray(x, dtype=np.float32)
    edge_index = np.asarray(edge_index)
    degree = np.asarray(degree, dtype=np.float32)
    N, D = x.shape
    H = np.asarray(Wb1).shape[1]

    plan = _plan(x, edge_index, degree)
    spc, n_win = plan["spc"], plan["n_win"]
    n_slots = spc * N_CORES
    node_of_slot = plan["node_of_slot"]

    # permuted node data (pad with zeros)
    x_pad = np.zeros((n_slots, D), np.float32)
    x_pad[: N] = x
    deg_pad = np.zeros(n_slots, np.float32)
    deg_pad[: N] = degree
    x_perm = x_pad[node_of_slot]          # [n_slots, D] rows in slot order
    deg_perm = deg_pad[node_of_slot]

    x_bf = x.astype(ml_dtypes.bfloat16)

    flags = dict(
        H=H,
        brb=bool(np.any(np.asarray(br) != 0)),
        bb2b=bool(np.any(np.asarray(bb2) != 0)),
        grbb=bool(np.any(np.asarray(g_rb) != 1)),
        brbb=bool(np.any(np.asarray(b_rb) != 0)),
        gnb=bool(np.any(np.asarray(g_n) != 1)),
        bnb=bool(np.any((np.asarray(b_n) + np.asarray(b2)) != 0)),
    )

    nc = _build_program(plan, flags)

    nH = H // P
    iota_arr = np.broadcast_to(np.arange(P, dtype=np.uint8)[None, :], (P, P)).copy()
    ident = np.eye(P, dtype=ml_dtypes.bfloat16)
    Wb2c = np.asarray(Wb2, np.float32).reshape(nH, P, P).transpose(1, 0, 2)
    W2c = np.asarray(W2, np.float32).reshape(nH, P, P).transpose(1, 0, 2)
    shared = {
        "xg": x_bf,
        "iota": iota_arr,
        "ident": ident,
        "Wr": np.asarray(Wr, np.float32).astype(ml_dtypes.bfloat16),
        "Wb1": np.asarray(Wb1, np.float32).astype(ml_dtypes.bfloat16),
        "W1": np.asarray(W1, np.float32).astype(ml_dtypes.bfloat16),
        "Wb2c": Wb2c.astype(ml_dtypes.bfloat16),
        "W2c": W2c.astype(ml_dtypes.bfloat16),
        "bb1c": (-np.asarray(bb1, np.float32)).reshape(nH, P).T.copy(),
        "b1c": np.asarray(b1, np.float32).reshape(nH, P).T.copy(),
    }
    if flags["brb"]:
        shared["brb"] = np.broadcast_to(np.asarray(br, np.float32)[None, :], (P, P)).copy()
    if flags["bb2b"]:
        shared["bb2b"] = np.broadcast_to(np.asarray(bb2, np.float32)[None, :], (P, P)).copy()
    if flags["grbb"]:
        shared["grbb"] = np.broadcast_to(np.asarray(g_rb, np.float32)[None, :], (P, P)).copy()
    if flags["brbb"]:
        shared["brbb"] = np.broadcast_to(np.asarray(b_rb, np.float32)[None, :], (P, P)).copy()
    if flags["gnb"]:
        shared["gnb"] = np.broadcast_to(np.asarray(g_n, np.float32)[None, :], (P, P)).copy()
    if flags["bnb"]:
        shared["bnb"] = np.broadcast_to(
            (np.asarray(b_n, np.float32) + np.asarray(b2, np.float32))[None, :],
            (P, P)).copy()

    in_maps = []
    for c in range(N_CORES):
        xc = x_perm[c * spc:(c + 1) * spc]
        m = dict(shared)
        m["xt"] = np.ascontiguousarray(xc.T).astype(ml_dtypes.bfloat16)
        m["xf"] = xc
        m["deg"] = np.ascontiguousarray(
            deg_perm[c * spc:(c + 1) * spc].reshape(n_win, P).T)
        m["idx"] = np.tile(
            plan["idx_streams"][c].reshape(-1, 16).T, (8, 1)).copy()
        m["slots"] = np.ascontiguousarray(plan["slots"][c].T)
        in_maps.append(m)

    global LAST_EXEC_TIME_NS
    if TRACE:
        _ensure_ntff_hook()
    res = run_bass_kernel_spmd(nc, in_maps, list(range(N_CORES)), trace=TRACE)
    LAST_EXEC_TIME_NS = res.exec_time_ns
    out_slots = np.concatenate([np.asarray(res.results[c]["out"])
                                for c in range(N_CORES)], axis=0)
    out = out_slots[plan["slot_of_node"][:N]]
    return out.astype(np.float32)


# revision 19
# speedup vs baseline: 1.0001x; 1.0001x over previous
"""Trainium2 Bass kernel for BoundaryConvLayer GNN message passing.

Strategy (8 NeuronCores, no collectives):
  - Nodes assigned to 8 cores x windows of 128 slots, balanced by in-degree.
  - x replicated (bf16) per core as DRAM gather table; each core DMA-gathers
    x[src] rows (256B) for its own edges via SWDGE dma_gather.  Gather streams
    are packed RAGGED per (8-window group, 32k src chunk): edges sorted by
    window, padded to 128 only at chunk-call boundaries (the SWDGE desc-gen at
    ~9.5ns/idx is the hard bottleneck, so padding is minimized).  Tile
    geometry (tile counts, window tile ranges) is the max/union over the 8
    cores so a single SPMD program serves all cores.
  - Scatter-reduce per dst window via one-hot matmuls: S built on DVE with
    is_equal against iota; boundary tiles shared by two windows use per-window
    masked slot streams (255 = no match).
  - Per-node MLPs in bf16; Softplus/Rsqrt/Gelu on ACT (batched per function to
    minimize ACT table loads); LayerNorm stats via bn_stats, applies on DVE.
  - Output written per-core, inverse-permuted on host.
"""

import sys

sys.path.insert(0, "/opt/trn_rl_repo")

import heapq

import ml_dtypes
import numpy as np

from concourse import bacc, bass, tile
from concourse.bass_utils import run_bass_kernel_spmd

mybir = bass.mybir
f32 = mybir.dt.float32
bf16 = mybir.dt.bfloat16
u8 = mybir.dt.uint8
i16 = mybir.dt.int16

P = 128
N_CORES = 8
TRACE = False
LAST_EXEC_TIME_NS = None
CHUNK = 32768  # int16 gather-index range per chunk
GROUP_SUPERS = 1  # supers (of up to 4 windows) per gather group
EPS = 1e-4
LN_EPS = 1e-5


# ----------------------------------------------------------------------------
# host-side planning
# ----------------------------------------------------------------------------

def _balanced_assignment(indeg, n_slots):
    """Assign node ids (len(indeg) <= n_slots) to n_slots//128 windows of 128
    slots each, minimizing max window edge-load. Returns slot_of_node."""
    n_win = n_slots // P
    n = len(indeg)
    order = np.argsort(-indeg, kind="stable")
    slot_of_node = np.empty(n_slots, dtype=np.int64)
    heap = [(0.0, w) for w in range(n_win)]
    heapq.heapify(heap)
    counts = np.zeros(n_win, dtype=np.int64)
    ids = np.concatenate([order, np.arange(n, n_slots)])
    degs = np.concatenate([indeg[order], np.zeros(n_slots - n, dtype=indeg.dtype)])
    for i in range(n_slots):
        while True:
            load, w = heapq.heappop(heap)
            if counts[w] < P:
                break
        slot_of_node[ids[i]] = w * P + counts[w]
        counts[w] += 1
        heapq.heappush(heap, (load + float(degs[i]), w))
    return slot_of_node


def _plan(x, edge_index, degree):
    N, D = x.shape
    assert D == P
    E = edge_index.shape[1]
    spc = -(-N // (N_CORES * P)) * P           # node slots per core
    n_win = spc // P                            # windows per core
    n_slots = spc * N_CORES
    NC = -(-N // CHUNK)                         # src chunks

    # supers (MLP tiles of up to 4 windows), groups (gather units of up to
    # GROUP_SUPERS supers)
    supers = []
    w0 = 0
    while w0 < n_win:
        W = min(4, n_win - w0)
        supers.append((w0, W))
        w0 += W
    groups = []
    si = 0
    while si < len(supers):
        gw0 = supers[si][0]
        nw = supers[si][1]
        sis = [si]
        for k in range(1, GROUP_SUPERS):
            if si + k < len(supers):
                nw += supers[si + k][1]
                sis.append(si + k)
        groups.append((gw0, nw, sis))
        si += GROUP_SUPERS
    NG = len(groups)
    grp_of_win = np.zeros(n_win, dtype=np.int64)
    for gi, (gw0, nw, _) in enumerate(groups):
        grp_of_win[gw0:gw0 + nw] = gi

    src = np.asarray(edge_index[0], dtype=np.int64)
    dst = np.asarray(edge_index[1], dtype=np.int64)
    indeg = np.bincount(dst, minlength=N)
    slot_of_node = _balanced_assignment(indeg, n_slots)
    node_of_slot = np.empty(n_slots, dtype=np.int64)
    node_of_slot[slot_of_node] = np.arange(n_slots)

    gslot = slot_of_node[dst]
    core = gslot // spc
    pos = gslot % spc
    gw = pos >> 7                    # window within core
    lane = pos & 127                 # dst slot within window
    grp = grp_of_win[gw]
    chunk = src // CHUNK
    srcloc = (src % CHUNK).astype(np.int16)

    # sort edges by (core, grp, chunk, window); ranks within (core, grp, chunk)
    key_gc = (core * NG + grp) * NC + chunk            # stream id
    order = np.lexsort((gw, chunk, grp, core))
    key_s = key_gc[order]
    first = np.ones(E, dtype=bool)
    first[1:] = key_s[1:] != key_s[:-1]
    starts = np.flatnonzero(first)
    seg_of = np.cumsum(first) - 1
    rank_s = np.arange(E) - starts[seg_of]             # rank in sorted order

    rank = np.empty(E, dtype=np.int64)
    rank[order] = rank_s

    # per (core, g, c) counts -> shared tile counts NT[g, c]
    cnt = np.bincount(key_gc, minlength=N_CORES * NG * NC).reshape(N_CORES, NG, NC)
    NT = -(-cnt.max(axis=0) // P)                      # [NG, NC]
    off = np.zeros((NG, NC), dtype=np.int64)
    for g in range(NG):
        o = 0
        for c in range(NC):
            off[g, c] = o
            o += NT[g, c]
    NT_g = NT.sum(axis=1)                              # [NG]
    idx_base = np.zeros(NG, dtype=np.int64)
    idx_base[1:] = np.cumsum(NT_g * P)[:-1]
    total_idx = int((NT_g * P).sum())

    # per (core, g, c, w) first/last rank -> union tile range over cores
    key_w = key_gc * n_win + gw
    key_w_s = key_w[order]
    uk, ufirst, ucnt = np.unique(key_w_s, return_index=True, return_counts=True)
    r_first = rank_s[ufirst]
    r_last = rank_s[ufirst + ucnt - 1]
    t0 = np.full((NG, NC, n_win), 2**30, dtype=np.int64)
    t1 = np.full((NG, NC, n_win), -1, dtype=np.int64)
    uw = uk % n_win
    ukgc = uk // n_win
    uc = ukgc % NC
    ug = (ukgc // NC) % NG
    np.minimum.at(t0, (ug, uc, uw), r_first >> 7)
    np.maximum.at(t1, (ug, uc, uw), r_last >> 7)

    # per-window matmul maps (G columns) and slot-stream layout
    gcols = [None] * n_win       # window -> list of G-tile column indices
    rw = np.zeros(n_win, dtype=np.int64)
    rng_off = np.zeros((NG, NC, n_win), dtype=np.int64)  # jj offset of (g,c,w)
    for w in range(n_win):
        g = grp_of_win[w]
        cols = []
        for c in range(NC):
            if t1[g, c, w] >= 0:
                rng_off[g, c, w] = len(cols)
                for t in range(t0[g, c, w], t1[g, c, w] + 1):
                    cols.append(off[g, c] + t)
            else:
                rng_off[g, c, w] = -1
        gcols[w] = cols
        rw[w] = len(cols)
    slot_base = np.zeros(n_win, dtype=np.int64)
    slot_base[1:] = np.cumsum(rw)[:-1]
    total_rw = int(rw.sum())

    # build per-core streams
    idx_streams = np.zeros((N_CORES, total_idx), dtype=np.int16)
    e_pos = idx_base[grp] + off[grp, chunk] * P + rank
    idx_streams[core, e_pos] = srcloc

    slots = np.full((N_CORES, total_rw, P), 255, dtype=np.uint8)
    e_jj = slot_base[gw] + rng_off[grp, chunk, gw] + (rank >> 7) - t0[grp, chunk, gw]
    slots[core, e_jj, rank & 127] = lane

    plan = dict(
        N=N, D=D, E=E, spc=spc, n_win=n_win, NC=NC, NG=NG,
        supers=supers, groups=groups,
        NT=NT, off=off, NT_g=NT_g, idx_base=idx_base,
        gcols=gcols, rw=rw, slot_base=slot_base,
        total_idx=total_idx, total_rw=total_rw,
        slot_of_node=slot_of_node, node_of_slot=node_of_slot,
        idx_streams=idx_streams, slots=slots,
    )
    return plan


# ----------------------------------------------------------------------------
# device program
# ----------------------------------------------------------------------------

def _build_program(plan, flags):
    n_win = plan["n_win"]
    NC, NG = plan["NC"], plan["NG"]
    supers, groups = plan["supers"], plan["groups"]
    NT, off, NT_g = plan["NT"], plan["off"], plan["NT_g"]
    idx_base = plan["idx_base"]
    gcols, rw, slot_base = plan["gcols"], plan["rw"], plan["slot_base"]
    spc = plan["spc"]
    N = plan["N"]
    total_idx, total_rw = plan["total_idx"], plan["total_rw"]
    H = flags["H"]
    nH = H // P

    NT_CAP = int(NT_g.max())
    RW_CAP = int(rw.max())
    # slots columns per super
    sup_sl = []
    for (w0, W) in supers:
        sup_sl.append((int(slot_base[w0]), int(rw[w0:w0 + W].sum())))
    SLW_CAP = max(s[1] for s in sup_sl)

    nc = bacc.Bacc("TRN2", target_bir_lowering=False)

    xg_d = nc.dram_tensor("xg", [N, P], bf16, kind="ExternalInput")
    xt_d = nc.dram_tensor("xt", [P, spc], bf16, kind="ExternalInput")
    xf_d = nc.dram_tensor("xf", [spc, P], f32, kind="ExternalInput")
    deg_d = nc.dram_tensor("deg", [P, n_win], f32, kind="ExternalInput")
    idx_d = nc.dram_tensor("idx", [P, total_idx // 16], i16, kind="ExternalInput")
    slots_d = nc.dram_tensor("slots", [P, total_rw], u8, kind="ExternalInput")
    iota_d = nc.dram_tensor("iota", [P, P], u8, kind="ExternalInput")
    Wr_d = nc.dram_tensor("Wr", [P, P], bf16, kind="ExternalInput")
    Wb1_d = nc.dram_tensor("Wb1", [P, H], bf16, kind="ExternalInput")
    W1_d = nc.dram_tensor("W1", [P, H], bf16, kind="ExternalInput")
    Wb2c_d = nc.dram_tensor("Wb2c", [P, nH, P], bf16, kind="ExternalInput")
    W2c_d = nc.dram_tensor("W2c", [P, nH, P], bf16, kind="ExternalInput")
    bb1c_d = nc.dram_tensor("bb1c", [P, nH], f32, kind="ExternalInput")
    b1c_d = nc.dram_tensor("b1c", [P, nH], f32, kind="ExternalInput")
    ident_d = nc.dram_tensor("ident", [P, P], bf16, kind="ExternalInput")
    bcast_names = [n for n in ("brb", "bb2b", "grbb", "brbb", "gnb", "bnb")
                   if flags[n]]
    bcast_d = {n: nc.dram_tensor(n, [P, P], f32, kind="ExternalInput")
               for n in bcast_names}
    out_d = nc.dram_tensor("out", [spc, P], f32, kind="ExternalOutput")

    AT = mybir.ActivationFunctionType
    OP = mybir.AluOpType

    with tile.TileContext(nc) as tc:
        with tc.tile_pool(name="const", bufs=1) as cp, \
             tc.tile_pool(name="stream", bufs=3) as sp, \
             tc.tile_pool(name="gat", bufs=3) as gp, \
             tc.tile_pool(name="sS", bufs=6) as ssp, \
             tc.tile_pool(name="work", bufs=2) as wp, \
             tc.tile_pool(name="tiny", bufs=8) as tp, \
             tc.tile_pool(name="ps_big", bufs=2, space="PSUM") as ps_big, \
             tc.tile_pool(name="ps_z", bufs=2, space="PSUM") as ps_z, \
             tc.tile_pool(name="ps_agg", bufs=2, space="PSUM") as ps_agg, \
             tc.tile_pool(name="ps_ht", bufs=1, space="PSUM") as ps_ht, \
             tc.tile_pool(name="ps_out", bufs=1, space="PSUM") as ps_out:

            # ---- constants
            def cload(dram, shape, dtype, tag):
                t = cp.tile(shape, dtype, tag=tag)
                nc.sync.dma_start(out=t[:], in_=dram[:])
                return t

            eps_t = cp.tile([P, 1], f32, tag="eps")
            nc.vector.memset(eps_t[:], LN_EPS)

            Wr_t = cload(Wr_d, [P, P], bf16, "Wr")
            Wb1_t = cload(Wb1_d, [P, H], bf16, "Wb1")
            W1_t = cload(W1_d, [P, H], bf16, "W1")
            Wb2c_t = cload(Wb2c_d, [P, nH, P], bf16, "Wb2c")
            W2c_t = cload(W2c_d, [P, nH, P], bf16, "W2c")
            bb1c_t = cload(bb1c_d, [P, nH], f32, "bb1c")
            b1c_t = cload(b1c_d, [P, nH], f32, "b1c")
            ident_t = cload(ident_d, [P, P], bf16, "ident")
            iota_t = cload(iota_d, [P, P], u8, "iota")
            deg_t = cload(deg_d, [P, n_win], f32, "deg")
            bcast_t = {n: cload(bcast_d[n], [P, P], f32, n) for n in bcast_names}

            for g, (gw0, gnw, sis) in enumerate(groups):
                ntg = int(NT_g[g])
                # ---- group streams: gather indices + gathered rows
                idx_t = sp.tile([P, NT_CAP * 8], i16, tag="idx")
                nc.sync.dma_start(
                    out=idx_t[:, :ntg * 8],
                    in_=idx_d[:, idx_base[g] // 16: (idx_base[g] + ntg * P) // 16])
                Gt = gp.tile([P, NT_CAP, P], bf16, tag="G")
                for c in range(NC):
                    n_t = int(NT[g, c])
                    if n_t == 0:
                        continue
                    lo = c * CHUNK
                    hi = min(N, lo + CHUNK)
                    o = int(off[g, c])
                    nc.gpsimd.dma_gather(
                        out_ap=Gt[:, o: o + n_t, :],
                        in_ap=xg_d[lo:hi, :],
                        idxs_ap=idx_t[:, o * 8: (o + n_t) * 8],
                        num_idxs=n_t * P, num_idxs_reg=n_t * P, elem_size=P,
                        single_packet=False)

                for si in sis:
                    w0, W = supers[si]
                    R = W * P
                    # ---- super streams
                    xt_s = sp.tile([P, 4 * P], bf16, tag="xt")
                    nc.sync.dma_start(out=xt_s[:, :R],
                                      in_=xt_d[:, w0 * P: w0 * P + R])
                    xf_s = sp.tile([P, 4, P], f32, tag="xf")
                    nc.sync.dma_start(
                        out=xf_s[:, :W, :],
                        in_=xf_d[w0 * P: w0 * P + R, :].rearrange(
                            "(w p) f -> p w f", p=P))
                    sl0, slw = sup_sl[si]
                    slots_t = sp.tile([P, SLW_CAP], u8, tag="slots")
                    nc.sync.dma_start(out=slots_t[:, :slw],
                                      in_=slots_d[:, sl0: sl0 + slw])

                    # ---- S one-hots (DVE) per window
                    Ss = []
                    so = 0
                    for wl in range(W):
                        rww = int(rw[w0 + wl])
                        S = ssp.tile([P, RW_CAP, P], bf16, tag="S")
                        if rww:
                            sb = slots_t[:, so: so + rww].broadcast_to(
                                [P, rww, P])
                            ib = iota_t[:].rearrange(
                                "p (o f) -> p o f", o=1).broadcast_to([P, rww, P])
                            nc.vector.tensor_tensor(out=S[:, :rww, :], in0=sb,
                                                    in1=ib, op=OP.is_equal)
                        Ss.append(S)
                        so += rww

                    # ---- rob_bound layer 1 (A-pattern) + rate.
                    # softplus(u) = -ln(sigmoid(-u)); we keep the NEGATED
                    # value (ln(sigmoid(-u))) and let the sign ride through.
                    # ACT sequence is batched by function: Sig x5, Ln x2.
                    g1T = wp.tile([P, nH, 4 * P], bf16, tag="g1T")
                    for cH in range(nH):
                        pb = ps_big.tile([P, 4 * P], f32, tag="bigA")
                        nc.tensor.matmul(pb[:, :R], Wb1_t[:, cH * P:(cH + 1) * P],
                                         xt_s[:, :R], start=True, stop=True)
                        nc.scalar.activation(g1T[:, cH, :R], pb[:, :R],
                                             AT.Sigmoid,
                                             bias=bb1c_t[:, cH:cH + 1],
                                             scale=-1.0)
                    # lr = ln(sigmoid(-(x@Wr+br))) = -rate
                    pr = ps_z.tile([P, 4, P], f32, tag="z")
                    for wl in range(W):
                        nc.tensor.matmul(pr[:, wl, :],
                                         xt_s[:, wl * P:(wl + 1) * P],
                                         Wr_t[:], start=True, stop=True)
                    lr = wp.tile([P, 4, P], f32, tag="rate")
                    if flags["brb"]:
                        br_b = bcast_t["brb"][:].rearrange(
                            "p (o f) -> p o f", o=1).broadcast_to([P, W, P])
                        nc.vector.tensor_tensor(out=lr[:, :W, :],
                                                in0=pr[:, :W, :], in1=br_b,
                                                op=OP.add)
                        nc.scalar.activation(lr[:, :W, :], lr[:, :W, :],
                                             AT.Sigmoid, bias=0.0, scale=-1.0)
                    else:
                        nc.scalar.activation(lr[:, :W, :], pr[:, :W, :],
                                             AT.Sigmoid, bias=0.0, scale=-1.0)
                    # Ln passes (one table load): g1T in-place, lr in-place
                    if R == 4 * P:
                        nc.scalar.activation(
                            g1T[:].rearrange("p c r -> p (c r)"),
                            g1T[:].rearrange("p c r -> p (c r)"),
                            AT.Ln, bias=0.0, scale=1.0)
                    else:
                        for cH in range(nH):
                            nc.scalar.activation(g1T[:, cH, :R], g1T[:, cH, :R],
                                                 AT.Ln, bias=0.0, scale=1.0)
                    nc.scalar.activation(lr[:, :W, :], lr[:, :W, :],
                                         AT.Ln, bias=0.0, scale=1.0)

                    # ---- rob_bound layer 2 (B-pattern) -> pz[rows, feat]
                    pz = ps_z.tile([P, 4, P], f32, tag="z")
                    for wl in range(W):
                        for cH in range(nH):
                            nc.tensor.matmul(
                                pz[:, wl, :],
                                g1T[:, cH, wl * P:(wl + 1) * P],
                                Wb2c_t[:, cH, :],
                                start=(cH == 0), stop=(cH == nH - 1))

                    z_in = pz
                    if flags["bb2b"]:
                        z_sb = wp.tile([P, 4, P], f32, tag="z_sb")
                        bb2_b = bcast_t["bb2b"][:].rearrange(
                            "p (o f) -> p o f", o=1).broadcast_to([P, W, P])
                        nc.vector.tensor_tensor(out=z_sb[:, :W, :],
                                                in0=pz[:, :W, :], in1=bb2_b,
                                                op=OP.subtract)
                        z_in = z_sb

                    # ---- LN stats (DVE) for z and x; rsqrt batched on ACT
                    mvz = tp.tile([P, 2 * 4], f32, tag="mvz")
                    mvx = tp.tile([P, 2 * 4], f32, tag="mvx")
                    for wl in range(W):
                        stz = tp.tile([P, 6], f32, tag="stz")
                        nc.vector.bn_stats(stz[:], z_in[:, wl, :])
                        nc.vector.bn_aggr(mvz[:, 2 * wl:2 * wl + 2], stz[:])
                        stx = tp.tile([P, 6], f32, tag="stx")
                        nc.vector.bn_stats(stx[:], xf_s[:, wl, :])
                        nc.vector.bn_aggr(mvx[:, 2 * wl:2 * wl + 2], stx[:])
                    sdz = tp.tile([P, 4], f32, tag="sdz")
                    sdx = tp.tile([P, 4], f32, tag="sdx")
                    for wl in range(W):
                        nc.scalar.activation(sdz[:, wl:wl + 1],
                                             mvz[:, 2 * wl + 1:2 * wl + 2],
                                             AT.Sqrt, bias=eps_t[:, 0:1],
                                             scale=1.0)
                        nc.scalar.activation(sdx[:, wl:wl + 1],
                                             mvx[:, 2 * wl + 1:2 * wl + 2],
                                             AT.Sqrt, bias=eps_t[:, 0:1],
                                             scale=1.0)
                    rsz = tp.tile([P, 4], f32, tag="rsz")
                    rsx = tp.tile([P, 4], f32, tag="rsx")
                    nc.vector.reciprocal(rsz[:, :W], sdz[:, :W])
                    nc.vector.reciprocal(rsx[:, :W], sdx[:, :W])
                    mbz = tp.tile([P, 4], f32, tag="mbz")
                    mbx = tp.tile([P, 4], f32, tag="mbx")
                    for wl in range(W):
                        nc.vector.tensor_scalar(
                            out=mbz[:, wl:wl + 1], in0=mvz[:, 2 * wl:2 * wl + 1],
                            scalar1=rsz[:, wl:wl + 1], scalar2=-1.0,
                            op0=OP.mult, op1=OP.mult)
                        nc.vector.tensor_scalar(
                            out=mbx[:, wl:wl + 1], in0=mvx[:, 2 * wl:2 * wl + 1],
                            scalar1=rsx[:, wl:wl + 1], scalar2=-1.0,
                            op0=OP.mult, op1=OP.mult)

                    # gamma' = -LN_core(z) (apply on DVE; pz holds -z so the
                    # normalized value comes out negated), then optional affine
                    gamma = wp.tile([P, 4, P], f32, tag="gamma")
                    for wl in range(W):
                        nc.vector.tensor_scalar(
                            out=gamma[:, wl, :], in0=z_in[:, wl, :],
                            scalar1=rsz[:, wl:wl + 1], scalar2=mbz[:, wl:wl + 1],
                            op0=OP.mult, op1=OP.add)
                    gamma_negated = True
                    if flags["grbb"]:
                        g_b = bcast_t["grbb"][:].rearrange(
                            "p (o f) -> p o f", o=1).broadcast_to([P, W, P])
                        nc.vector.scalar_tensor_tensor(
                            out=gamma[:, :W, :], in0=gamma[:, :W, :],
                            scalar=-1.0, in1=g_b, op0=OP.mult, op1=OP.mult)
                        gamma_negated = False
                    if flags["brbb"]:
                        b_b = bcast_t["brbb"][:].rearrange(
                            "p (o f) -> p o f", o=1).broadcast_to([P, W, P])
                        if gamma_negated:
                            nc.vector.scalar_tensor_tensor(
                                out=gamma[:, :W, :], in0=gamma[:, :W, :],
                                scalar=-1.0, in1=b_b, op0=OP.mult, op1=OP.add)
                            gamma_negated = False
                        else:
                            nc.vector.tensor_tensor(out=gamma[:, :W, :],
                                                    in0=gamma[:, :W, :],
                                                    in1=b_b, op=OP.add)

                    # ---- agg: one-hot matmul accumulation per window
                    pagg = ps_agg.tile([P, 4, P], f32, tag="agg")
                    for wl in range(W):
                        cols = gcols[w0 + wl]
                        nj = len(cols)
                        if nj == 0:
                            nc.vector.memset(pagg[:, wl, :], 0.0)
                            continue
                        S = Ss[wl]
                        for jj, gcol in enumerate(cols):
                            nc.tensor.matmul(
                                pagg[:, wl, :],
                                S[:, jj, :],
                                Gt[:, gcol, :],
                                start=(jj == 0), stop=(jj == nj - 1))

                    # ---- h = (rate*agg + gamma) / (1 + rate*deg + EPS)
                    # rate = -lr ; gamma = -gamma' when gamma_negated
                    num = wp.tile([P, 4, P], f32, tag="num")
                    nc.vector.scalar_tensor_tensor(
                        out=num[:, :W, :], in0=lr[:, :W, :], scalar=-1.0,
                        in1=pagg[:, :W, :], op0=OP.mult, op1=OP.mult)
                    nc.vector.tensor_tensor(
                        out=num[:, :W, :], in0=num[:, :W, :],
                        in1=gamma[:, :W, :],
                        op=OP.subtract if gamma_negated else OP.add)
                    den = wp.tile([P, 4, P], f32, tag="den")
                    deg_b = deg_t[:, w0:w0 + W].rearrange(
                        "p (w o) -> p w o", o=1).broadcast_to([P, W, P])
                    nc.vector.scalar_tensor_tensor(
                        out=den[:, :W, :], in0=lr[:, :W, :], scalar=-1.0,
                        in1=deg_b, op0=OP.mult, op1=OP.mult)
                    nc.vector.tensor_scalar(out=den[:, :W, :],
                                            in0=den[:, :W, :],
                                            scalar1=1.0 + EPS, scalar2=None,
                                            op0=OP.add)
                    rcp = wp.tile([P, 4, P], f32, tag="rcp")
                    nc.vector.reciprocal_approx_fast(out=rcp[:, :W, :],
                                                     in_=den[:, :W, :])
                    h_bf = wp.tile([P, 4, P], bf16, tag="h_bf")
                    nc.vector.tensor_tensor(out=h_bf[:, :W, :],
                                            in0=num[:, :W, :],
                                            in1=rcp[:, :W, :], op=OP.mult)

                    # ---- hT via PE transpose
                    pht = ps_ht.tile([P, 4, P], bf16, tag="ht")
                    for wl in range(W):
                        nc.tensor.transpose(pht[:, wl, :], h_bf[:, wl, :],
                                            ident_t[:])
                    hT = wp.tile([P, 4 * P], bf16, tag="hT")
                    nc.vector.tensor_copy(
                        hT[:, :R], pht[:, :W, :].rearrange("p w f -> p (w f)"))

                    # ---- fc layer 1 (A-pattern on hT) + gelu
                    g2T = wp.tile([P, nH, 4 * P], bf16, tag="g2T")
                    for cH in range(nH):
                        pf = ps_big.tile([P, 4 * P], f32, tag="bigA")
                        nc.tensor.matmul(pf[:, :R], W1_t[:, cH * P:(cH + 1) * P],
                                         hT[:, :R], start=True, stop=True)
                        nc.scalar.activation(g2T[:, cH, :R], pf[:, :R], AT.Gelu,
                                             bias=b1c_t[:, cH:cH + 1], scale=1.0)

                    # ---- fc layer 2 (B-pattern) -> po[rows, feat]
                    po = ps_out.tile([P, 4, P], f32, tag="o")
                    for wl in range(W):
                        for cH in range(nH):
                            nc.tensor.matmul(
                                po[:, wl, :],
                                g2T[:, cH, wl * P:(wl + 1) * P],
                                W2c_t[:, cH, :],
                                start=(cH == 0), stop=(cH == nH - 1))

                    # ---- x_res = LN(x) applied on DVE; out = po + x_res
                    xres = wp.tile([P, 4, P], f32, tag="xres")
                    for wl in range(W):
                        nc.vector.tensor_scalar(
                            out=xres[:, wl, :], in0=xf_s[:, wl, :],
                            scalar1=rsx[:, wl:wl + 1], scalar2=mbx[:, wl:wl + 1],
                            op0=OP.mult, op1=OP.add)
                    if flags["gnb"]:
                        g_b = bcast_t["gnb"][:].rearrange(
                            "p (o f) -> p o f", o=1).broadcast_to([P, W, P])
                        nc.vector.tensor_tensor(out=xres[:, :W, :],
                                                in0=xres[:, :W, :], in1=g_b,
                                                op=OP.mult)
                    if flags["bnb"]:
                        b_b = bcast_t["bnb"][:].rearrange(
                            "p (o f) -> p o f", o=1).broadcast_to([P, W, P])
                        nc.vector.tensor_tensor(out=xres[:, :W, :],
                                                in0=xres[:, :W, :], in1=b_b,
                                                op=OP.add)

                    out_sb = wp.tile([P, 4, P], f32, tag="out_sb")
                    nc.vector.tensor_tensor(out=out_sb[:, :W, :],
                                            in0=po[:, :W, :],
                                            in1=xres[:, :W, :], op=OP.add)
                    nc.sync.dma_start(
                        out=out_d[w0 * P: w0 * P + R, :].rearrange(
                            "(w p) f -> p w f", p=P),
                        in_=out_sb[:, :W, :])

    nc.compile()
    return nc


def _ensure_ntff_hook():
    """Register the axon NTFF profile hook when the container's antenv
    package lacks axon_hooks (needed for trace=True under axon)."""
    import types
    try:
        from antenv.axon_hooks import get_axon_ntff_profile_hook  # noqa: F401
        return
    except ImportError:
        pass
    if "/root/.axon_site" not in sys.path:
        sys.path.insert(0, "/root/.axon_site")
    from trn_agent_boot.trn_boot import _ntff_profile_via_ctypes
    import antenv
    hook = _ntff_profile_via_ctypes("/opt/axon/libaxon_pjrt.so")
    mod = types.ModuleType("antenv.axon_hooks")
    mod.get_axon_ntff_profile_hook = lambda: hook
    mod.set_axon_ntff_profile_hook = lambda h: None
    sys.modules["antenv.axon_hooks"] = mod
    antenv.axon_hooks = mod


# ----------------------------------------------------------------------------
# entry point
# ----------------------------------------------------------------------------

def kernel(x, edge_index, degree, Wr, br, Wb1, bb1, Wb2, bb2, g_rb, b_rb,
           W1, b1, W2, b2, g_n, b_n):
    x = np.asarray(x, dtype=np.float32)
    edge_index = np.asarray(edge_index)
    degree = np.asarray(degree, dtype=np.float32)
    N, D = x.shape
    H = np.asarray(Wb1).shape[1]

    plan = _plan(x, edge_index, degree)
    spc, n_win = plan["spc"], plan["n_win"]
    n_slots = spc * N_CORES
    node_of_slot = plan["node_of_slot"]

    # permuted node data (pad with zeros)
    x_pad = np.zeros((n_slots, D), np.float32)
    x_pad[: N] = x
    deg_pad = np.zeros(n_slots, np.float32)
    deg_pad[: N] = degree
    x_perm = x_pad[node_of_slot]          # [n_slots, D] rows in slot order
    deg_perm = deg_pad[node_of_slot]

    x_bf = x.astype(ml_dtypes.bfloat16)

    flags = dict(
        H=H,
        brb=bool(np.any(np.asarray(br) != 0)),
        bb2b=bool(np.any(np.asarray(bb2) != 0)),
        grbb=bool(np.any(np.asarray(g_rb) != 1)),
        brbb=bool(np.any(np.asarray(b_rb) != 0)),
        gnb=bool(np.any(np.asarray(g_n) != 1)),
        bnb=bool(np.any((np.asarray(b_n) + np.asarray(b2)) != 0)),
    )

    nc = _build_program(plan, flags)

    nH = H // P
    iota_arr = np.broadcast_to(np.arange(P, dtype=np.uint8)[None, :], (P, P)).copy()
    ident = np.eye(P, dtype=ml_dtypes.bfloat16)
    Wb2c = np.asarray(Wb2, np.float32).reshape(nH, P, P).transpose(1, 0, 2)
    W2c = np.asarray(W2, np.float32).reshape(nH, P, P).transpose(1, 0, 2)
    shared = {
        "xg": x_bf,
        "iota": iota_arr,
        "ident": ident,
        "Wr": np.asarray(Wr, np.float32).astype(ml_dtypes.bfloat16),
        "Wb1": np.asarray(Wb1, np.float32).astype(ml_dtypes.bfloat16),
        "W1": np.asarray(W1, np.float32).astype(ml_dtypes.bfloat16),
        "Wb2c": Wb2c.astype(ml_dtypes.bfloat16),
        "W2c": W2c.astype(ml_dtypes.bfloat16),
        "bb1c": (-np.asarray(bb1, np.float32)).reshape(nH, P).T.copy(),
        "b1c": np.asarray(b1, np.float32).reshape(nH, P).T.copy(),
    }
    if flags["brb"]:
        shared["brb"] = np.broadcast_to(np.asarray(br, np.float32)[None, :], (P, P)).copy()
    if flags["bb2b"]:
        shared["bb2b"] = np.broadcast_to(np.asarray(bb2, np.float32)[None, :], (P, P)).copy()
    if flags["grbb"]:
        shared["grbb"] = np.broadcast_to(np.asarray(g_rb, np.float32)[None, :], (P, P)).copy()
    if flags["brbb"]:
        shared["brbb"] = np.broadcast_to(np.asarray(b_rb, np.float32)[None, :], (P, P)).copy()
    if flags["gnb"]:
        shared["gnb"] = np.broadcast_to(np.asarray(g_n, np.float32)[None, :], (P, P)).copy()
    if flags["bnb"]:
        shared["bnb"] = np.broadcast_to(
            (np.asarray(b_n, np.float32) + np.asarray(b2, np.float32))[None, :],
            (P, P)).copy()

    in_maps = []
    for c in range(N_CORES):
        xc = x_perm[c * spc:(c + 1) * spc]
        m = dict(shared)
        m["xt"] = np.ascontiguousarray(xc.T).astype(ml_dtypes.bfloat16)
        m["xf"] = xc
        m["deg"] = np.ascontiguousarray(
            deg_perm[c * spc:(c + 1) * spc].reshape(n_win, P).T)
        m["idx"] = np.tile(
            plan["idx_streams"][c].reshape(-1, 16).T, (8, 1)).copy()
        m["slots"] = np.ascontiguousarray(plan["slots"][c].T)
        in_maps.append(m)

    global LAST_EXEC_TIME_NS
    if TRACE:
        _ensure_ntff_hook()
    res = run_bass_kernel_spmd(nc, in_maps, list(range(N_CORES)), trace=TRACE)
    LAST_EXEC_TIME_NS = res.exec_time_ns
    out_slots = np.concatenate([np.asarray(res.results[c]["out"])
                                for c in range(N_CORES)], axis=0)
    out = out_slots[plan["slot_of_node"][:N]]
    return out.astype(np.float32)
